# revision 1
# baseline (speedup 1.0000x reference)
"""Multi-head attention (B=8, N=1024, C=768, H=12) on 8 TRN2 NeuronCores.

Data-parallel over batch: core b computes batch element b end-to-end.

Per-core dataflow (matmul operands in fp16 — 1 cycle/row on the PE, fp32
PSUM accumulation; measured end-to-end relative error ~7e-4):

  qkT[f,t]  = wqkT.T @ xT          (q,k kept feature-major for the S matmul)
  v[t,f]    = xT.T @ wvT           (token-major, padded with a ones column
                                    per head so the PV matmul also produces
                                    the softmax denominator)
  ST[kj,qi] = kT.T @ qT            (per head, K=64; two 512-wide matmuls
                                    into one 2-bank PSUM tile)
  E         = exp(SCALE * ST)      (ScalarE on [128,1024] tiles — amortizes
                                    the ~352-cycle ACTIVATE overhead; no
                                    max-subtraction: |S|<9, well in fp16)
  oT'[d+1,qi] = [v_h|1].T @ E      (accumulated over kj chunks; row 64 holds
                                    the denominator colsum)
  oT_h      = oT'[0:64] / bcast(colsum)     (rank-1 PE broadcast + DVE div)
  outT[f,t] = woT.T @ oT + b_out   (bias via per-partition tensor_scalar add,
                                    fp32 all the way to the output)

Phase 2 is software-pipelined per head: ST/exp of head h are emitted before
PV/normalize of head h-1, keeping the PE busy while ScalarE works through
the exps (PE gaps > ~3.4us de-warm the HAM clock gate to 1.2 GHz).

Host side casts x/weights to fp16 in the layouts above and transposes the
fp32 outT result back.
"""

import os

import numpy as np

import concourse.bass as bass
import concourse.tile as tile
from concourse import mybir
from concourse.bass_utils import run_bass_kernel_spmd

B, N, C = 8, 1024, 768
H, D = 12, 64
SCALE = D ** -0.5
CC = C // 128          # 6 contraction chunks
QKF = (2 * C) // 128   # 12 q|k feature chunks
NT2 = N // 512         # 2 free-dim chunks of 512
NT8 = N // 128         # 8 partition chunks of 128
F32 = mybir.dt.float32
F16 = mybir.dt.float16

# normalize via DVE tensor_tensor divide (True) or reciprocal+mul (False).
# divide is NOT a valid DVE ALU op on TRN2 (s3s3d3_tt_valid_op) — keep False.
NORM_DIVIDE = False

_N_CORES = 8


def _split_multiwaits(nc, max_waits: int = 1):
    """The pinned walrus codegen supports one embedded sync-wait per engine
    instruction (single EVENTS slot in the TPB ISA).  Tile's tail drain /
    barriers accumulate several; hoist all-but-one wait onto same-engine
    NoOps placed immediately before the instruction (waits AND, so order is
    irrelevant)."""
    n_split = 0
    for f in nc.m.functions:
        for blk in f.blocks:
            insts = blk.instructions
            if not any(
                ins.sync_info is not None and len(ins.sync_info.on_wait) > max_waits
                for ins in insts
            ):
                continue
            new_list = []
            for ins in insts:
                si = ins.sync_info
                if si is not None and len(si.on_wait) > max_waits:
                    waits = list(si.on_wait)
                    hoist, keep = waits[:-max_waits], waits[-max_waits:]
                    for w in hoist:
                        nop = mybir.InstNoOp(name=nc.get_next_instruction_name())
                        nop.engine = ins.engine
                        nop.sync_info = mybir.SyncInfo(on_wait=[w], on_update=[])
                        new_list.append(nop)
                        n_split += 1
                    ins.sync_info = mybir.SyncInfo(
                        on_wait=keep, on_update=list(si.on_update)
                    )
                new_list.append(ins)
            blk.instructions = new_list
    return n_split


def _build(split: bool = True):
    nc = bass.Bass()
    xT = nc.declare_dram_parameter("xT", [C, N], F16, isOutput=False)
    wqkT = nc.declare_dram_parameter("wqkT", [C, 2 * C], F16, isOutput=False)
    wvT = nc.declare_dram_parameter("wvT", [C, C], F16, isOutput=False)
    woT = nc.declare_dram_parameter("woT", [C, C], F16, isOutput=False)
    bo = nc.declare_dram_parameter("bo", [C, 1], F32, isOutput=False)
    outT = nc.declare_dram_parameter("outT", [C, N], F32, isOutput=True)

    with tile.TileContext(nc) as tc:
        with (
            tc.tile_pool(name="sb", bufs=1) as sb,
            tc.tile_pool(name="psum", bufs=1, space="PSUM") as psum,
        ):
            qkT = [
                sb.tile([128, N], F16, tag=f"qkT{j}", name=f"qkT{j}")
                for j in range(QKF)
            ]
            v_sb = [
                sb.tile([128, H * (D + 1)], F16, tag=f"v{t}", name=f"v{t}")
                for t in range(NT8)
            ]
            oT = [sb.tile([128, N], F16, tag=f"oT{c}", name=f"oT{c}") for c in range(CC)]
            bo_t = [sb.tile([128, 1], F32, tag=f"bo{c}", name=f"bo{c}") for c in range(CC)]
            xr = [sb.tile([128, N], F16, tag=f"xr{c}", name=f"xr{c}") for c in range(CC)]
            wqk = [
                sb.tile([128, 2 * C], F16, tag=f"wqk{c}", name=f"wqk{c}")
                for c in range(CC)
            ]
            wv = [sb.tile([128, C], F16, tag=f"wv{c}", name=f"wv{c}") for c in range(CC)]
            wo = [sb.tile([128, C], F16, tag=f"wo{c}", name=f"wo{c}") for c in range(CC)]

            for c in range(CC):
                sl = slice(c * 128, (c + 1) * 128)
                nc.sync.dma_start(out=xr[c], in_=xT[sl, :])
                nc.sync.dma_start(out=wqk[c], in_=wqkT[sl, :])
                nc.sync.dma_start(out=wv[c], in_=wvT[sl, :])
                nc.sync.dma_start(out=wo[c], in_=woT[sl, :])
                nc.sync.dma_start(out=bo_t[c], in_=bo[sl, :])

            ones12 = sb.tile([128, H], F16, tag="ones12")
            nc.vector.memset(ones12, 1.0)
            ones1h = sb.tile([1, D], F16, tag="ones1h")
            nc.vector.memset(ones1h, 1.0)

            # ---------------- phase 1: projections ----------------
            # 1a: qkT[j] = sum_c wqk[c][:, j-block].T @ xr[c]   (both 512-halves
            # of the token dim accumulate into one 2-bank PSUM tile)
            for j in range(QKF):
                p = psum.tile([128, N], F32, tag="big", name="big", bufs=3)
                for c in range(CC):
                    for t2 in range(NT2):
                        nc.tensor.matmul(
                            p[:, t2 * 512 : (t2 + 1) * 512],
                            wqk[c][:, j * 128 : (j + 1) * 128],
                            xr[c][:, t2 * 512 : (t2 + 1) * 512],
                            start=(c == 0),
                            stop=(c == CC - 1),
                        )
                nc.vector.tensor_copy(qkT[j], p)

            # 1b: v[t][:, h*65:h*65+64] = sum_c xr[c][:, t-block].T @ wv[c]
            for t8 in range(NT8):
                # two 384-wide halves at offsets 0 and 512 (a matmul may not
                # cross a PSUM bank boundary)
                p = psum.tile([128, N], F32, tag="big", name="big", bufs=3)
                for c in range(CC):
                    for nh in range(2):
                        nc.tensor.matmul(
                            p[:, nh * 512 : nh * 512 + 384],
                            xr[c][:, t8 * 128 : (t8 + 1) * 128],
                            wv[c][:, nh * 384 : (nh + 1) * 384],
                            start=(c == 0),
                            stop=(c == CC - 1),
                        )
                v_view = v_sb[t8].rearrange("p (h e) -> p h e", e=D + 1)
                for nh in range(2):
                    nc.vector.tensor_copy(
                        v_view[:, nh * 6 : (nh + 1) * 6, 0:D],
                        p[:, nh * 512 : nh * 512 + 384].rearrange(
                            "p (h d) -> p h d", d=D
                        ),
                    )
                nc.vector.tensor_copy(v_view[:, :, D : D + 1], ones12.unsqueeze(2))

            # ---------------- phase 2: attention (SW-pipelined per head) ----
            # ST/exp of head h are interleaved per-kj-chunk with PV of head
            # h-1 so the in-order PE queue never idles long enough to de-warm
            # the HAM clock gate while ScalarE works through the exps.
            with tc.tile_pool(name="ph2", bufs=12) as ph2, tc.tile_pool(
                name="ph2s", bufs=4
            ) as ph2s:

                def emit_st(h, kc):
                    kt = qkT[CC + h // 2]
                    qt = qkT[h // 2]
                    po = (h % 2) * 64
                    st = psum.tile([128, N], F32, tag="big", name="big", bufs=3)
                    for t2 in range(NT2):
                        nc.tensor.matmul(
                            st[:, t2 * 512 : (t2 + 1) * 512],
                            kt[po : po + D, kc * 128 : (kc + 1) * 128],
                            qt[po : po + D, t2 * 512 : (t2 + 1) * 512],
                            start=True,
                            stop=True,
                        )
                    e = ph2.tile([128, N], F16, tag="exps", name="exps")
                    nc.scalar.activation(
                        e, st, mybir.ActivationFunctionType.Exp, scale=SCALE
                    )
                    return e

                def emit_pv(h, ex, ov, kc):
                    for t2 in range(NT2):
                        nc.tensor.matmul(
                            ov[t2][0 : D + 1, :],
                            v_sb[kc][:, h * (D + 1) : (h + 1) * (D + 1)],
                            ex[kc][:, t2 * 512 : (t2 + 1) * 512],
                            start=(kc == 0),
                            stop=(kc == NT8 - 1),
                        )

                def emit_norm(h, ov):
                    # ov[t2] layout: rows 0:64 = head output, row 64 = colsum,
                    # rows 64:128 reused for the rank-1 recip broadcast
                    po = (h % 2) * 64
                    for t2 in range(NT2):
                        o = ov[t2]
                        qs = slice(t2 * 512, (t2 + 1) * 512)
                        rec = ph2s.tile([1, 512], F32, tag="rec", name="rec")
                        nc.vector.reciprocal(rec, o[D : D + 1, :])
                        rec16 = ph2s.tile([1, 512], F16, tag="rec16", name="rec16")
                        nc.vector.tensor_copy(rec16, rec)
                        nc.tensor.matmul(
                            o[D : D + 64, :], ones1h, rec16, start=True, stop=True
                        )
                        bc_sb = ph2s.tile([D, 512], F32, tag="bcsb", name="bcsb")
                        nc.vector.tensor_copy(bc_sb, o[D : D + 64, :])
                        nc.vector.tensor_mul(
                            oT[h // 2][po : po + D, qs], o[0:D, :], bc_sb
                        )

                def new_ov():
                    return [
                        psum.tile([128, 512], F32, tag="ops", name="ops", bufs=2)
                        for _ in range(NT2)
                    ]

                prev_ex = None
                prev_ov = None
                for h in range(H):
                    ex = []
                    ov = new_ov() if h > 0 else None
                    for kc in range(NT8):
                        ex.append(emit_st(h, kc))
                        if h > 0:
                            emit_pv(h - 1, prev_ex, ov, kc)
                    if h > 0:
                        emit_norm(h - 1, ov)
                    prev_ex = ex
                ov = new_ov()
                for kc in range(NT8):
                    emit_pv(H - 1, prev_ex, ov, kc)
                emit_norm(H - 1, ov)

            # ---------------- phase 3: output projection ----------------
            with tc.tile_pool(name="ph3o", bufs=3) as ph3o:
                for fc in range(CC):
                    p = psum.tile([128, N], F32, tag="big", name="big", bufs=3)
                    for c in range(CC):
                        for t2 in range(NT2):
                            nc.tensor.matmul(
                                p[:, t2 * 512 : (t2 + 1) * 512],
                                wo[c][:, fc * 128 : (fc + 1) * 128],
                                oT[c][:, t2 * 512 : (t2 + 1) * 512],
                                start=(c == 0),
                                stop=(c == CC - 1),
                            )
                    ot = ph3o.tile([128, N], F32, tag="outsb", name="outsb")
                    nc.vector.tensor_scalar_add(ot, p, bo_t[fc])
                    nc.sync.dma_start(
                        out=outT[fc * 128 : (fc + 1) * 128, :], in_=ot
                    )

    if split:
        _split_multiwaits(nc)
    return nc


_NC = None


def _get_nc():
    global _NC
    if _NC is None:
        _NC = _build()
    return _NC


def kernel(x, w_qkv, w_out, b_out):
    x = np.asarray(x, dtype=np.float32)
    w_qkv = np.asarray(w_qkv, dtype=np.float32)
    w_out = np.asarray(w_out, dtype=np.float32)
    b_out = np.asarray(b_out, dtype=np.float32)

    wqkT = np.ascontiguousarray(w_qkv[: 2 * C].T.astype(np.float16))
    wvT = np.ascontiguousarray(w_qkv[2 * C :].T.astype(np.float16))
    woT = np.ascontiguousarray(w_out.T.astype(np.float16))
    bo = np.ascontiguousarray(b_out.reshape(C, 1))

    in_maps = [
        {
            "xT": np.ascontiguousarray(x[b].T.astype(np.float16)),
            "wqkT": wqkT,
            "wvT": wvT,
            "woT": woT,
            "bo": bo,
        }
        for b in range(B)
    ]

    nc = _get_nc()
    trace = bool(os.environ.get("KERNEL_TRACE"))
    res = run_bass_kernel_spmd(nc, in_maps, list(range(_N_CORES)), trace=trace)
    if trace:
        print(f"HW exec time: {res.exec_time_ns} ns")
        if res.instructions_and_trace is not None:
            print(f"trace: {res.instructions_and_trace[1]}")

    out = np.empty((B, N, C), dtype=np.float32)
    for b in range(B):
        out[b] = res.results[b]["outT"].T
    return out



# revision 36
# speedup vs baseline: 1.4826x; 1.4826x over previous
"""Multi-head attention (B=8, N=1024, C=768, H=12) on 8 TRN2 NeuronCores.

Data-parallel over batch: core b computes batch element b end-to-end.

Per-core dataflow (matmul operands in fp16 — 1 cycle/row on the PE, fp32
PSUM accumulation; measured end-to-end relative error ~7e-4):

  qkT[f,t]  = wqkT.T @ xT          (q,k kept feature-major for the S matmul)
  v[t,f]    = xT.T @ wvT           (token-major, padded with a ones column
                                    per head so the PV matmul also produces
                                    the softmax denominator)
  ST[kj,qi] = kT.T @ qT            (per head, K=64; two 512-wide matmuls
                                    into one 2-bank PSUM tile)
  E         = exp(SCALE * ST)      (ScalarE on [128,1024] tiles — amortizes
                                    the ~352-cycle ACTIVATE overhead; no
                                    max-subtraction: |S|<9, well in fp16)
  oT'[d+1,qi] = [v_h|1].T @ E      (accumulated over kj chunks; row 64 holds
                                    the denominator colsum)
  oT_h      = oT'[0:64] / bcast(colsum)     (rank-1 PE broadcast + DVE div)
  outT[f,t] = woT.T @ oT + b_out   (bias via per-partition tensor_scalar add,
                                    fp32 all the way to the output)

Phase 2 is software-pipelined per head: ST/exp of head h are emitted before
PV/normalize of head h-1, keeping the PE busy while ScalarE works through
the exps (PE gaps > ~3.4us de-warm the HAM clock gate to 1.2 GHz).

Host side casts x/weights to fp16 in the layouts above and transposes the
fp32 outT result back.
"""

import os

import numpy as np

import concourse.bass as bass
import concourse.tile as tile
from concourse import mybir
from concourse.bass_utils import run_bass_kernel_spmd

B, N, C = 8, 1024, 768
H, D = 12, 64
SCALE = D ** -0.5
# subtracted from the scaled logits before exp so that the unnormalized
# PV outputs (up to ~denominator * |v|) stay well inside fp16 range; the
# shift cancels exactly in the softmax normalization.
EXP_BIAS = -5.0
CC = C // 128          # 6 contraction chunks
QKF = (2 * C) // 128   # 12 q|k feature chunks
NT2 = N // 512         # 2 free-dim chunks of 512
NT8 = N // 128         # 8 partition chunks of 128
F32 = mybir.dt.float32
F16 = mybir.dt.float16

# normalize via DVE tensor_tensor divide (True) or reciprocal+mul (False).
# divide is NOT a valid DVE ALU op on TRN2 (s3s3d3_tt_valid_op) — keep False.
NORM_DIVIDE = False

_N_CORES = 8


def _split_multiwaits(nc, max_waits: int = 1):
    """The pinned walrus codegen supports one embedded sync-wait per engine
    instruction (single EVENTS slot in the TPB ISA).  Tile's tail drain /
    barriers accumulate several; hoist all-but-one wait onto same-engine
    NoOps placed immediately before the instruction (waits AND, so order is
    irrelevant)."""
    n_split = 0
    for f in nc.m.functions:
        for blk in f.blocks:
            insts = blk.instructions
            if not any(
                ins.sync_info is not None and len(ins.sync_info.on_wait) > max_waits
                for ins in insts
            ):
                continue
            new_list = []
            for ins in insts:
                si = ins.sync_info
                if si is not None and len(si.on_wait) > max_waits:
                    waits = list(si.on_wait)
                    hoist, keep = waits[:-max_waits], waits[-max_waits:]
                    for w in hoist:
                        nop = mybir.InstNoOp(name=nc.get_next_instruction_name())
                        nop.engine = ins.engine
                        nop.sync_info = mybir.SyncInfo(on_wait=[w], on_update=[])
                        new_list.append(nop)
                        n_split += 1
                    ins.sync_info = mybir.SyncInfo(
                        on_wait=keep, on_update=list(si.on_update)
                    )
                new_list.append(ins)
            blk.instructions = new_list
    return n_split


def _build(split: bool = True):
    nc = bass.Bass()
    xT = nc.declare_dram_parameter("xT", [C, N], F16, isOutput=False)
    wqkT = nc.declare_dram_parameter("wqkT", [C, 2 * C], F16, isOutput=False)
    wvT = nc.declare_dram_parameter("wvT", [C, C], F16, isOutput=False)
    woT = nc.declare_dram_parameter("woT", [C, C], F16, isOutput=False)
    bo = nc.declare_dram_parameter("bo", [C, 1], F32, isOutput=False)
    indA_d = nc.declare_dram_parameter("indA", [8, 4 * 128], F16, isOutput=False)
    indB_d = nc.declare_dram_parameter("indB", [4, 2 * 128], F16, isOutput=False)
    outT = nc.declare_dram_parameter("outT", [C, N], F32, isOutput=True)

    with tile.TileContext(nc) as tc:
        with (
            tc.tile_pool(name="sb", bufs=1) as sb,
            tc.tile_pool(name="psum", bufs=1, space="PSUM") as psum,
        ):
            qkT = [
                sb.tile([128, N], F16, tag=f"qkT{j}", name=f"qkT{j}")
                for j in range(QKF)
            ]
            v_sb = [
                sb.tile([128, H * (D + 1)], F16, tag=f"v{t}", name=f"v{t}")
                for t in range(NT8)
            ]
            oT = [sb.tile([128, N], F16, tag=f"oT{c}", name=f"oT{c}") for c in range(CC)]
            bo_t = [sb.tile([128, 1], F32, tag=f"bo{c}", name=f"bo{c}") for c in range(CC)]
            xr = [sb.tile([128, N], F16, tag=f"xr{c}", name=f"xr{c}") for c in range(CC)]
            wqk = [
                sb.tile([128, 2 * C], F16, tag=f"wqk{c}", name=f"wqk{c}")
                for c in range(CC)
            ]
            wv = [sb.tile([128, C], F16, tag=f"wv{c}", name=f"wv{c}") for c in range(CC)]
            wo = [sb.tile([128, C], F16, tag=f"wo{c}", name=f"wo{c}") for c in range(CC)]

            for c in range(CC):
                sl = slice(c * 128, (c + 1) * 128)
                nc.sync.dma_start(out=xr[c], in_=xT[sl, :])
                nc.sync.dma_start(out=wqk[c], in_=wqkT[sl, :])
                nc.sync.dma_start(out=wv[c], in_=wvT[sl, :])
                nc.sync.dma_start(out=wo[c], in_=woT[sl, :])
                nc.sync.dma_start(out=bo_t[c], in_=bo[sl, :])

            # per-partition bias operand for the exp activation
            ebias = sb.tile([128, 1], F32, tag="ebias")
            nc.vector.memset(ebias, EXP_BIAS)
            ones12 = sb.tile([128, H], F16, tag="ones12")
            nc.vector.memset(ones12, 1.0)

            # softmax denominators, assembled head-major via sbuf->sbuf DMA
            # (engine APs need 32-aligned partition bases; DMA does not).
            # Split 8+4 so the first batched reciprocal can run as soon as
            # heads 0-7 have drained.
            dallA = sb.tile([8, N], F32, tag="dallA")
            dallB = sb.tile([4, N], F32, tag="dallB")
            recA = sb.tile([8, N], F32, tag="recA")
            recB = sb.tile([4, N], F32, tag="recB")
            rec16A = sb.tile([8, N], F16, tag="rec16A")
            rec16B = sb.tile([4, N], F16, tag="rec16B")
            indA_t = sb.tile([8, 4 * 128], F16, tag="indA")
            indB_t = sb.tile([4, 2 * 128], F16, tag="indB")
            nc.sync.dma_start(out=indA_t, in_=indA_d[:, :])
            nc.sync.dma_start(out=indB_t, in_=indB_d[:, :])
            r16 = [
                sb.tile([128, N], F16, tag=f"r16_{c}", name=f"r16_{c}")
                for c in range(CC)
            ]

            # ---------------- phase 1: projections ----------------
            # 1a: qkT[j] = sum_c wqk[c][:, j-block].T @ xr[c]   (both 512-halves
            # of the token dim accumulate into one 2-bank PSUM tile)
            for j in range(QKF):
                p = psum.tile([128, N], F32, tag="big", name="big", bufs=3)
                for c in range(CC):
                    for t2 in range(NT2):
                        nc.tensor.matmul(
                            p[:, t2 * 512 : (t2 + 1) * 512],
                            wqk[c][:, j * 128 : (j + 1) * 128],
                            xr[c][:, t2 * 512 : (t2 + 1) * 512],
                            start=(c == 0),
                            stop=(c == CC - 1),
                        )
                nc.vector.tensor_copy(qkT[j], p)

            # 1b: v[t][:, h*65:h*65+64] = sum_c xr[c][:, t-block].T @ wv[c]
            for t8 in range(NT8):
                # two 384-wide halves at offsets 0 and 512 (a matmul may not
                # cross a PSUM bank boundary)
                p = psum.tile([128, N], F32, tag="big", name="big", bufs=3)
                for c in range(CC):
                    for nh in range(2):
                        nc.tensor.matmul(
                            p[:, nh * 512 : nh * 512 + 384],
                            xr[c][:, t8 * 128 : (t8 + 1) * 128],
                            wv[c][:, nh * 384 : (nh + 1) * 384],
                            start=(c == 0),
                            stop=(c == CC - 1),
                        )
                v_view = v_sb[t8].rearrange("p (h e) -> p h e", e=D + 1)
                for nh in range(2):
                    nc.vector.tensor_copy(
                        v_view[:, nh * 6 : (nh + 1) * 6, 0:D],
                        p[:, nh * 512 : nh * 512 + 384].rearrange(
                            "p (h d) -> p h d", d=D
                        ),
                    )
                nc.vector.tensor_copy(v_view[:, :, D : D + 1], ones12.unsqueeze(2))

            # ---------------- phase 2: attention (SW-pipelined per head) ----
            # ST/exp of head h are interleaved per-kj-chunk with PV of head
            # h-1 so the in-order PE queue never idles long enough to de-warm
            # the HAM clock gate while ScalarE works through the exps.
            with tc.tile_pool(name="ph2", bufs=12) as ph2, tc.tile_pool(
                name="ph2s", bufs=4
            ) as ph2s:

                def emit_st(h, kc):
                    kt = qkT[CC + h // 2]
                    qt = qkT[h // 2]
                    po = (h % 2) * 64
                    st = psum.tile([128, N], F32, tag="big", name="big", bufs=3)
                    for t2 in range(NT2):
                        nc.tensor.matmul(
                            st[:, t2 * 512 : (t2 + 1) * 512],
                            kt[po : po + D, kc * 128 : (kc + 1) * 128],
                            qt[po : po + D, t2 * 512 : (t2 + 1) * 512],
                            start=True,
                            stop=True,
                        )
                    e = ph2.tile([128, N], F16, tag="exps", name="exps")
                    nc.scalar.activation(
                        e, st, mybir.ActivationFunctionType.Exp, scale=SCALE,
                        bias=ebias,
                    )
                    return e

                def emit_pv(h, ex, ov, kc):
                    for t2 in range(NT2):
                        nc.tensor.matmul(
                            ov[t2][0 : D + 1, :],
                            v_sb[kc][:, h * (D + 1) : (h + 1) * (D + 1)],
                            ex[kc][:, t2 * 512 : (t2 + 1) * 512],
                            start=(kc == 0),
                            stop=(kc == NT8 - 1),
                        )

                def emit_drain(h, ov):
                    # stash unnormalized head output + denominator row; the
                    # reciprocal is batched over many heads (a [1,512] DVE
                    # reciprocal is free-size-bound: 24 of them cost 95us on
                    # one lane in the baseline)
                    po = (h % 2) * 64
                    dtile, row = (dallA, h) if h < 8 else (dallB, h - 8)
                    for t2 in range(NT2):
                        o = ov[t2]
                        qs = slice(t2 * 512, (t2 + 1) * 512)
                        nc.vector.tensor_copy(oT[h // 2][po : po + D, qs], o[0:D, :])
                        stg = ph2s.tile([1, 512], F32, tag="stg", name="stg")
                        nc.vector.tensor_copy(stg, o[D : D + 1, :])
                        nc.sync.dma_start(out=dtile[row : row + 1, qs], in_=stg)

                def emit_recip_r(dtile, rtile, r16tile, ind_t, cs):
                    # batched reciprocal + indicator-matmul broadcast of the
                    # per-head reciprocals over their 64-feature blocks,
                    # then normalize the oT chunks of head pairs `cs`
                    nc.vector.reciprocal(rtile, dtile)
                    nc.vector.tensor_copy(r16tile, rtile)
                    for i, c in enumerate(cs):
                        p = psum.tile([128, N], F32, tag="big", name="big", bufs=3)
                        for t2 in range(NT2):
                            nc.tensor.matmul(
                                p[:, t2 * 512 : (t2 + 1) * 512],
                                ind_t[:, i * 128 : (i + 1) * 128],
                                r16tile[:, t2 * 512 : (t2 + 1) * 512],
                                start=True,
                                stop=True,
                            )
                        nc.vector.tensor_copy(r16[c], p)
                        nc.vector.tensor_mul(oT[c], oT[c], r16[c])

                def new_ov():
                    return [
                        psum.tile([128, 512], F32, tag="ops", name="ops", bufs=2)
                        for _ in range(NT2)
                    ]

                prev_ex = None
                for h in range(H):
                    ex = []
                    ov = new_ov() if h > 0 else None
                    for kc in range(NT8):
                        ex.append(emit_st(h, kc))
                        if h > 0:
                            emit_pv(h - 1, prev_ex, ov, kc)
                    if h > 0:
                        emit_drain(h - 1, ov)
                    if h == 9:
                        # heads 0-7 drained: hide the first reciprocal +
                        # broadcast under the remaining exps
                        emit_recip_r(dallA, recA, rec16A, indA_t, (0, 1, 2, 3))
                    prev_ex = ex
                ov = new_ov()
                for kc in range(NT8):
                    emit_pv(H - 1, prev_ex, ov, kc)
                emit_drain(H - 1, ov)
                emit_recip_r(dallB, recB, rec16B, indB_t, (4, 5))

            # ---------------- phase 3: output projection ----------------
            with tc.tile_pool(name="ph3o", bufs=3) as ph3o:
                for fc in range(CC):
                    p = psum.tile([128, N], F32, tag="big", name="big", bufs=3)
                    for c in range(CC):
                        for t2 in range(NT2):
                            nc.tensor.matmul(
                                p[:, t2 * 512 : (t2 + 1) * 512],
                                wo[c][:, fc * 128 : (fc + 1) * 128],
                                oT[c][:, t2 * 512 : (t2 + 1) * 512],
                                start=(c == 0),
                                stop=(c == CC - 1),
                            )
                    ot = ph3o.tile([128, N], F32, tag="outsb", name="outsb")
                    nc.vector.tensor_scalar_add(ot, p, bo_t[fc])
                    nc.sync.dma_start(
                        out=outT[fc * 128 : (fc + 1) * 128, :], in_=ot
                    )

    if split:
        _split_multiwaits(nc)
    return nc


_NC = None


def _get_nc():
    global _NC
    if _NC is None:
        _NC = _build()
    return _NC


def kernel(x, w_qkv, w_out, b_out):
    x = np.asarray(x, dtype=np.float32)
    w_qkv = np.asarray(w_qkv, dtype=np.float32)
    w_out = np.asarray(w_out, dtype=np.float32)
    b_out = np.asarray(b_out, dtype=np.float32)

    wqkT = np.ascontiguousarray(w_qkv[: 2 * C].T.astype(np.float16))
    wvT = np.ascontiguousarray(w_qkv[2 * C :].T.astype(np.float16))
    woT = np.ascontiguousarray(w_out.T.astype(np.float16))
    bo = np.ascontiguousarray(b_out.reshape(C, 1))
    # indicator matrices: indA[r, c*128 + f] = 1 iff head 2c + f//64 == r
    indA = np.zeros((8, 4 * 128), dtype=np.float16)
    indB = np.zeros((4, 2 * 128), dtype=np.float16)
    for c in range(4):
        indA[2 * c, c * 128 : c * 128 + D] = 1.0
        indA[2 * c + 1, c * 128 + D : (c + 1) * 128] = 1.0
    for c in range(2):
        indB[2 * c, c * 128 : c * 128 + D] = 1.0
        indB[2 * c + 1, c * 128 + D : (c + 1) * 128] = 1.0

    in_maps = [
        {
            "xT": np.ascontiguousarray(x[b].T.astype(np.float16)),
            "wqkT": wqkT,
            "wvT": wvT,
            "woT": woT,
            "bo": bo,
            "indA": indA,
            "indB": indB,
        }
        for b in range(B)
    ]

    nc = _get_nc()
    trace = bool(os.environ.get("KERNEL_TRACE"))
    res = run_bass_kernel_spmd(nc, in_maps, list(range(_N_CORES)), trace=trace)
    if trace:
        print(f"HW exec time: {res.exec_time_ns} ns")
        if res.instructions_and_trace is not None:
            print(f"trace: {res.instructions_and_trace[1]}")

    out = np.empty((B, N, C), dtype=np.float32)
    for b in range(B):
        out[b] = res.results[b]["outT"].T
    return out

